# revision 4
# baseline (speedup 1.0000x reference)
"""ConfidenceBiasedCrossAttention Trainium2 kernel (8 NeuronCores), v2.

Sharding (Megatron-style): data-parallel over B (2) x head-parallel over
4 head-groups of 4 heads (256 channels) -> 8 cores. Host sums the 8
partial outputs (2 head-pair partials per core) per batch and adds the
bias (Wo_b + Wv_b @ Wo_w.T; the V-projection bias folds out because
softmax weights sum to exactly 1, even under quantization, since the
denominator uses the same quantized weights via the ones column).

Key design:
  - HOST pre-transposes activations and weights and casts them to bf16
    (QT/KT/VT = x^T so the contraction dim is on partitions): the device
    runs ZERO transposes, and DMA volume halves (~24MB/core).
  - All matmuls bf16 (1 col/cycle, same PE rate as f32r) into f32 PSUM.
  - qT[ch,Lq], kT[ch,Lk] projections with weight-chunk stationary;
    v natural [Lk,ch] with x-chunk stationary / WvT moving; extra ones
    column in v65 produces the softmax denominator inside attn@v.
  - logits chunk [128k, Lq] = kT_h^T qT_h (K=64, tile_position);
    ACT: eT = exp(0.125*logits + V_bias[k]) -> bf16. ACT is the tail
    bottleneck (~1.07us per 128x1024 exp, 128 exps).
  - heads 0,1 stream behind the K/V projection blocks; heads 2,3 run
    after, with head-pair-0's output projection interleaved; partial
    outputs are written per head-pair (bf16) and summed on host.

PSUM budget (8 banks): pl [128,1024]x2 (projections/logits/bcast/Wo,
2 banks each) + acc [128,1024]x2 (two live attn accumulators, 2 banks
each) = 8.
"""

import numpy as np
import ml_dtypes

import concourse.bacc as bacc
import concourse.mybir as mybir
import concourse.tile as tile
from concourse import bass_utils

F32 = mybir.dt.float32
F32R = mybir.dt.float32r
BF16 = mybir.dt.bfloat16
FP8 = mybir.dt.float8e4
AF = mybir.ActivationFunctionType
MUL = mybir.AluOpType.mult
DR = mybir.MatmulPerfMode.DoubleRow
NP_BF16 = ml_dtypes.bfloat16
NP_FP8 = ml_dtypes.float8_e4m3fn

# fp8 q/k for the logits matmul (DoubleRow, 2x PE rate). The host permutes
# Wq/Wk output channels so the projection writes the [32-part x 2-ktile]
# layout DoubleRow wants; logits are invariant to a shared q/k channel
# permutation. Measured end-to-end error ~1.2e-2 vs the 2e-2 gate.
USE_FP8_QK = True

P = 128
C = 1024
D = 64
LQ = 1024
LK = 4096
CS = 256          # channels per core (4 heads)
NH = 4            # heads per core
SCALE = 1.0 / 8.0
BLK = 512         # Lk rows per processing block
NBLK = LK // BLK  # 8
NCH = LK // P     # 32 Lk chunks of 128
QB = 512          # Lq rows per Q-projection block


def build_nc():
    nc = bacc.Bacc("TRN2", target_bir_lowering=False, debug=False, num_devices=8)
    qt_d = nc.dram_tensor("QT", [C, LQ], BF16, kind="ExternalInput").ap()
    kt_d = nc.dram_tensor("KT", [C, LK], BF16, kind="ExternalInput").ap()
    vt_d = nc.dram_tensor("VT", [C, LK], BF16, kind="ExternalInput").ap()
    vbias_d = nc.dram_tensor("vbias", [P, NCH], F32, kind="ExternalInput").ap()
    wq_d = nc.dram_tensor("wq", [C, CS], BF16, kind="ExternalInput").ap()
    wk_d = nc.dram_tensor("wk", [C, CS], BF16, kind="ExternalInput").ap()
    wv_d = nc.dram_tensor("wv", [C, CS], BF16, kind="ExternalInput").ap()
    wo_d = nc.dram_tensor("wo", [CS, C], BF16, kind="ExternalInput").ap()
    bq_d = nc.dram_tensor("bq", [P, 2], F32, kind="ExternalInput").ap()
    bk_d = nc.dram_tensor("bk", [P, 2], F32, kind="ExternalInput").ap()
    out_d = nc.dram_tensor("out", [2, LQ, C], BF16, kind="ExternalOutput").ap()

    with tile.TileContext(nc) as tc:
        with (
            tc.tile_pool(name="pers", bufs=1) as pers,
            tc.tile_pool(name="sb", bufs=1) as sb,
            tc.tile_pool(name="ps", bufs=2, space="PSUM") as ps,
        ):
            # ---- constants ----
            ones_f32 = pers.tile([P, 1], F32)
            nc.gpsimd.memset(ones_f32, 1.0)
            ones_r = pers.tile([1, D], F32R)
            nc.vector.tensor_copy(ones_r, ones_f32[0:1, :].to_broadcast([1, D]))
            vbias_sb = pers.tile([P, NCH], F32)
            nc.sync.dma_start(vbias_sb, vbias_d)
            bq_sb = pers.tile([P, 2], F32)
            nc.sync.dma_start(bq_sb, bq_d)
            bk_sb = pers.tile([P, 2], F32)
            nc.sync.dma_start(bk_sb, bk_d)

            # ---- persistent activations ----
            QKDT = FP8 if USE_FP8_QK else BF16
            qT = pers.tile([P, 2, LQ], QKDT)       # [ch%128, ch//128, Lq]
            kT = pers.tile([P, 2, LK], QKDT)
            if USE_FP8_QK:
                # DoubleRow layout: [32*hl+pp, hh, t, L]; head h=(hh=h//2,
                # hl=h%2) strip sits at base partition 32*hl (0/32), with
                # channel d = 32*t + pp. Filled by SBUF->SBUF DMA remap.
                qT8 = pers.tile([D, 2, 2, LQ], FP8)
                kT8 = pers.tile([D, 2, 2, LK], FP8)

                def remap(dst8, stage, lo, hi):
                    # 4 simple DMAs (3-dim APs): strip (hl,t) of the stage
                    # moves to partitions 32*hl..+32, ktile column t
                    for hl in range(2):
                        for t in range(2):
                            p0 = hl * 64 + t * 32
                            nc.gpsimd.dma_start(
                                dst8[hl * 32 : (hl + 1) * 32, :, t, lo:hi],
                                stage[p0 : p0 + 32, :, lo:hi],
                            )
            v65 = pers.tile([P, NCH, NH, D + 1], BF16)  # [k%128, chunk, h, v|1]
            attnT = pers.tile([P, 2, LQ], BF16)
            wqT = pers.tile([P, 8, CS], BF16)      # [cin%128, cin//128, cout]
            wkT = pers.tile([P, 8, CS], BF16)
            wvT = pers.tile([P, 8, CS], BF16)
            woT = pers.tile([P, 2, C], BF16)       # [ch%128, ch//128, cout]
            RING = 24
            e2ring = pers.tile([P, RING, LQ], BF16)  # head-2 attn weights

            # ones column of v65 (softmax denominator trick)
            with nc.allow_low_precision(reason="ones column exact in bf16"):
                nc.vector.tensor_copy(
                    v65[:, :, :, D].rearrange("p a b -> p (a b)"),
                    ones_f32.to_broadcast([P, NCH * NH]),
                )

            def issue_kv(blk):
                kx = sb.tile([P, 8, BLK], BF16, tag="kx", bufs=2)
                nc.sync.dma_start(
                    kx,
                    kt_d[:, blk * BLK : (blk + 1) * BLK].rearrange(
                        "(i p) r -> p i r", p=P
                    ),
                )
                vx = sb.tile([P, 8, BLK], BF16, tag="vx", bufs=2)
                nc.sync.dma_start(
                    vx,
                    vt_d[:, blk * BLK : (blk + 1) * BLK].rearrange(
                        "(i p) r -> p i r", p=P
                    ),
                )
                return kx, vx

            # ---- DMA order: wk, K+V block 0, wq, wv, Q halves ----
            nc.sync.dma_start(wkT, wk_d.rearrange("(i p) o -> p i o", p=P))
            pend = {0: issue_kv(0)}
            nc.sync.dma_start(wqT, wq_d.rearrange("(i p) o -> p i o", p=P))
            nc.sync.dma_start(wvT, wv_d.rearrange("(i p) o -> p i o", p=P))

            def q_proj():
                # emitted AFTER block-0 projections so PE's in-order queue
                # starts on K/V work (whose DMAs land first)
                for h in range(2):
                    qx = sb.tile([P, 8, QB], BF16, tag="qx", bufs=2)
                    nc.sync.dma_start(
                        qx,
                        qt_d[:, h * QB : (h + 1) * QB].rearrange(
                            "(i p) r -> p i r", p=P
                        ),
                    )
                    for mt in range(2):
                        pq = ps.tile([P, LQ], F32, tag="pl", bufs=2, name="pq")
                        for i in range(8):
                            nc.tensor.matmul(
                                pq[:, 0:QB], wqT[:, i, mt * P : (mt + 1) * P],
                                qx[:, i, :], start=(i == 0), stop=(i == 7),
                            )
                        with nc.allow_low_precision(reason="bf16 activations"):
                            nc.vector.tensor_scalar_add(
                                qT[:, mt, h * QB : (h + 1) * QB], pq[:, 0:QB],
                                bq_sb[:, mt : mt + 1],
                            )
                    if USE_FP8_QK:
                        remap(qT8, qT, h * QB, (h + 1) * QB)

            # ---- attention helpers ----
            def attn_logits(h, c, dst=None):
                ht, hp = h // 2, (h % 2) * D
                pl = ps.tile([P, LQ], F32, tag="pl", bufs=2, name="pl")
                hh, hl = h // 2, (h % 2) * 32
                for n in range(2):
                    if USE_FP8_QK:
                        nc.tensor.matmul(
                            pl[:, n * 512 : (n + 1) * 512],
                            kT8[hl : hl + 32, hh, :, c * P : (c + 1) * P],
                            qT8[hl : hl + 32, hh, :, n * 512 : (n + 1) * 512],
                            start=True, stop=True, perf_mode=DR,
                        )
                    else:
                        nc.tensor.matmul(
                            pl[:, n * 512 : (n + 1) * 512],
                            kT[hp : hp + D, ht, c * P : (c + 1) * P],
                            qT[hp : hp + D, ht, n * 512 : (n + 1) * 512],
                            start=True, stop=True, tile_position=(hp, 0),
                        )
                eT = dst if dst is not None else sb.tile(
                    [P, LQ], BF16, tag="exp", bufs=4, name="eT"
                )
                with nc.allow_low_precision(reason="attn weights bf16"):
                    nc.scalar.activation(
                        eT, pl, AF.Exp, bias=vbias_sb[:, c : c + 1], scale=SCALE
                    )
                return eT

            def attn_av(h, c, po, eT):
                for n in range(2):
                    nc.tensor.matmul(
                        po[0 : D + 1, n * 512 : (n + 1) * 512],
                        v65[:, c, h, :],
                        eT[:, n * 512 : (n + 1) * 512],
                        start=(c == 0), stop=(c == NCH - 1),
                    )

            def attn_chunk(h, c, po):
                attn_av(h, c, po, attn_logits(h, c))

            def attn_finish(h, po):
                ht, hp = h // 2, (h % 2) * D
                rec = sb.tile([1, LQ], F32R, tag="rec", bufs=2, name="rec")
                with nc.allow_low_precision(reason="softmax denom reciprocal"):
                    nc.vector.reciprocal(rec, po[D : D + 1, :])
                pb = ps.tile([P, LQ], F32, tag="pl", bufs=2, name="pb")
                for n in range(2):
                    nc.tensor.matmul(
                        pb[0:D, n * 512 : (n + 1) * 512],
                        ones_r,
                        rec[:, n * 512 : (n + 1) * 512],
                        start=True, stop=True,
                    )
                bc = sb.tile([D, LQ], F32, tag="bc", bufs=2, name="bc")
                nc.scalar.copy(bc, pb[0:D, :])
                with nc.allow_low_precision(reason="bf16 attn output"):
                    nc.vector.tensor_tensor(
                        attnT[hp : hp + D, ht, :], po[0:D, :], bc, MUL
                    )

            def out_proj_m(kc, m, eng=0, split=False):
                pw = ps.tile([P, C], F32, tag="pl", bufs=2, name="pw")
                for n in range(2):
                    if split:  # per-head halves (even head's half ready first)
                        for hp in range(2):
                            nc.tensor.matmul(
                                pw[:, n * 512 : (n + 1) * 512],
                                attnT[hp * D : (hp + 1) * D, kc,
                                      m * P : (m + 1) * P],
                                woT[hp * D : (hp + 1) * D, kc,
                                    n * 512 : (n + 1) * 512],
                                start=(hp == 0), stop=(hp == 1),
                                tile_position=(hp * D, 0),
                            )
                    else:
                        nc.tensor.matmul(
                            pw[:, n * 512 : (n + 1) * 512],
                            attnT[:, kc, m * P : (m + 1) * P],
                            woT[:, kc, n * 512 : (n + 1) * 512],
                            start=True, stop=True,
                        )
                ob = sb.tile([P, C], BF16, tag="ob", bufs=4)
                with nc.allow_low_precision(reason="bf16 partial out"):
                    if eng:
                        nc.scalar.copy(ob, pw)
                    else:
                        nc.vector.tensor_copy(ob, pw)
                nc.sync.dma_start(out_d[kc, m * P : (m + 1) * P, :], ob)

            # ---- K/V projection blocks; heads 0-1 stream behind them ----
            po0 = ps.tile([P, LQ], F32, tag="acc", bufs=2, name="po0")
            po1 = ps.tile([P, LQ], F32, tag="acc", bufs=2, name="po1")
            sprev = None

            def stream_attn(blk):
                nonlocal sprev
                for cc in range(4):
                    c = blk * 4 + cc
                    e0 = attn_logits(0, c)
                    e1 = attn_logits(1, c)
                    if sprev is not None:
                        attn_av(0, sprev[0], po0, sprev[1])
                        attn_av(1, sprev[0], po1, sprev[2])
                    sprev = (c, e0, e1)
                    if cc < 3:  # precompute head-2 exps (3 per block)
                        c2 = blk * 3 + cc
                        attn_logits(2, c2, dst=e2ring[:, c2, :])

            for blk in range(NBLK):
                kx, vx = pend.pop(blk)
                if blk == NBLK - 1:
                    nc.sync.dma_start(
                        woT, wo_d.rearrange("(t p) c -> p t c", p=P)
                    )
                for mt in range(2):
                    pk = ps.tile([P, LQ], F32, tag="pl", bufs=2, name="pk")
                    for i in range(8):
                        nc.tensor.matmul(
                            pk[:, 0:BLK], wkT[:, i, mt * P : (mt + 1) * P],
                            kx[:, i, :], start=(i == 0), stop=(i == 7),
                        )
                    with nc.allow_low_precision(reason="bf16 activations"):
                        nc.vector.tensor_scalar_add(
                            kT[:, mt, blk * BLK : (blk + 1) * BLK], pk[:, 0:BLK],
                            bk_sb[:, mt : mt + 1],
                        )
                if USE_FP8_QK:
                    remap(kT8, kT, blk * BLK, (blk + 1) * BLK)
                for pr in range(2):  # pairs of 128-row chunks
                    pv = ps.tile([P, LQ], F32, tag="pl", bufs=2, name="pv")
                    for half in range(2):
                        lo = pr * 256 + half * P
                        for i in range(8):
                            nc.tensor.matmul(
                                pv[:, half * 256 : (half + 1) * 256],
                                vx[:, i, lo : lo + P], wvT[:, i, :],
                                start=(i == 0), stop=(i == 7),
                            )
                    c0 = blk * 4 + pr * 2
                    with nc.allow_low_precision(reason="v bf16"):
                        nc.vector.tensor_copy(
                            v65[:, c0 : c0 + 2, :, 0:D],
                            pv[:, 0:BLK].rearrange("p (r h d) -> p r h d", r=2, d=D),
                        )
                if blk == 0:
                    q_proj()
                if blk + 1 < NBLK:
                    pend[blk + 1] = issue_kv(blk + 1)
                # attention one block behind: hides proj->remap latency
                if blk > 0:
                    stream_attn(blk - 1)
            stream_attn(NBLK - 1)
            attn_av(0, sprev[0], po0, sprev[1])
            attn_av(1, sprev[0], po1, sprev[2])
            attn_finish(0, po0)
            attn_finish(1, po1)

            # ---- heads 2-3 (kT/v65 complete); head-pair-0 out streams ----
            po2 = ps.tile([P, LQ], F32, tag="acc", bufs=2, name="po2")
            po3 = ps.tile([P, LQ], F32, tag="acc", bufs=2, name="po3")
            # one-chunk software pipeline: emit logits(c) before attn@v(c-1)
            # so the exp stream never waits on PE
            prev = None
            for c in range(NCH):
                e3 = attn_logits(3, c)
                e2 = e2ring[:, c, :] if c < RING else attn_logits(2, c)
                if prev is not None:
                    attn_av(2, prev[0], po2, prev[1])
                    attn_av(3, prev[0], po3, prev[2])
                prev = (c, e2, e3)
                if c % 4 == 2:
                    out_proj_m(0, c // 4)
            attn_av(2, prev[0], po2, prev[1])
            attn_av(3, prev[0], po3, prev[2])
            attn_finish(2, po2)
            attn_finish(3, po3)
            for m in range(8):
                out_proj_m(1, m, eng=(m % 2))

    nc.compile()
    return nc


_NC = None


def _get_nc():
    global _NC
    if _NC is None:
        _NC = build_nc()
    return _NC


def _bf(x):
    return np.ascontiguousarray(x.astype(NP_BF16))




def shard_inputs(Q, K_in, V_in, V_bias, Wq_w, Wq_b, Wk_w, Wk_b, Wv_w, Wv_b, Wo_w, Wo_b):
    """Build the 8 per-core input dicts (host pre-transposes + casts bf16)."""
    per_batch = []
    for b in range(2):
        per_batch.append({
            "QT": _bf(np.asarray(Q[b]).T),
            "KT": _bf(np.asarray(K_in[b]).T),
            "VT": _bf(np.asarray(V_in[b]).T),
            "vbias": np.ascontiguousarray(np.asarray(V_bias[b]).reshape(NCH, P).T),
        })
    in_maps = []
    for core in range(8):
        b, g = core // 4, core % 4
        gs, ge = g * CS, (g + 1) * CS
        in_maps.append({
            **per_batch[b],
            "wq": _bf(np.asarray(Wq_w)[gs:ge].T),
            "wk": _bf(np.asarray(Wk_w)[gs:ge].T),
            "wv": _bf(np.asarray(Wv_w)[gs:ge].T),
            "wo": _bf(np.asarray(Wo_w)[:, gs:ge].T),
            "bq": np.ascontiguousarray(np.asarray(Wq_b)[gs:ge].reshape(2, P).T),
            "bk": np.ascontiguousarray(np.asarray(Wk_b)[gs:ge].reshape(2, P).T),
        })
    return in_maps


def combine_outputs(results, Wv_b, Wo_w, Wo_b):
    """Sum the 2x4 head-group partials per batch; add output bias and the
    folded V-projection bias (attention weights sum to 1)."""
    bias = Wo_b + Wv_b @ Wo_w.T
    outs = np.stack(
        [np.asarray(r["out"]).astype(np.float32) for r in results]
    ).reshape(2, 4 * 2, LQ, C)
    return (outs.sum(axis=1) + bias[None, None, :]).astype(np.float32)


def kernel(**inputs):
    nc = _get_nc()
    in_maps = shard_inputs(**inputs)
    res = bass_utils.run_bass_kernel_spmd(nc, in_maps, core_ids=list(range(8)))
    return combine_outputs(
        res.results,
        np.asarray(inputs["Wv_b"]),
        np.asarray(inputs["Wo_w"]),
        np.asarray(inputs["Wo_b"]),
    )


if __name__ == "__main__":
    rng = np.random.default_rng(0)
    ins = {
        "Q": rng.standard_normal((2, LQ, C), dtype=np.float32),
        "K_in": rng.standard_normal((2, LK, C), dtype=np.float32),
        "V_in": rng.standard_normal((2, LK, C), dtype=np.float32),
        "V_bias": rng.standard_normal((2, LK)).astype(np.float32),
        **{
            f"W{x}_w": (rng.standard_normal((C, C)) * 0.03).astype(np.float32)
            for x in "qkvo"
        },
        **{
            f"W{x}_b": (rng.standard_normal(C) * 0.03).astype(np.float32)
            for x in "qkvo"
        },
    }
    out = kernel(**ins)
    print("ok", out.shape, out.dtype)


# revision 5
# speedup vs baseline: 5.1277x; 5.1277x over previous
"""ConfidenceBiasedCrossAttention Trainium2 kernel (8 NeuronCores), v3.

Sharding (Megatron-style): data-parallel over B (2) x head-parallel over
4 head-groups of 4 heads (256 channels) -> 8 cores. Host sums the 8
partial outputs (2 head-pair partials per core) per batch and adds the
bias (Wo_b + Wv_b @ Wo_w.T; the V-projection bias folds out because
softmax weights sum to exactly 1, even under quantization, since the
denominator uses the same quantized weights via the ones column).

Key design:
  - HOST pre-transposes activations and weights and casts them to bf16
    (QT/KT/VT = x^T so the contraction dim is on partitions): the device
    runs ZERO transposes, and DMA volume halves (~24MB/core).
  - Projections in bf16 (1 col/cycle) into f32 PSUM; q/k are then cast
    to fp8e4m3 and SBUF->SBUF DMA-remapped (idle SWDGE/Pool queue) into
    a [32*hl+pp, hh, ktile, L] layout whose per-head strips sit at legal
    base partitions 0/32, enabling DoubleRow matmuls for the logits at
    0.5 cycles/col (2x PE rate). End-to-end rel err 1.23e-2 (gate 2e-2);
    the v/attention-weight path stays bf16 (fp8 there fails the gate).
  - logits chunk [128k, Lq] = kT8_h^T qT8_h (DoubleRow, K=2x32);
    ACT: eT = exp(0.125*logits + V_bias[k]) -> bf16; attn@v via the
    [v|1] ones-column trick accumulates numerator+denominator in PSUM.
  - heads 0,1 stream one block behind the K/V projections; head 2's
    first 24 exp chunks are precomputed into an SBUF ring during the
    (ACT-idle) streaming phase; heads 2,3 + head-pair-0's output
    projection fill the tail; partial outputs are written per head-pair
    (bf16) and summed on host.

PSUM budget (8 banks): pl [128,1024]x2 (projections/logits/bcast/Wo,
2 banks each) + acc [128,1024]x2 (two live attn accumulators, 2 banks
each) = 8.

TimelineSim: 237.6us/core (baseline kernel: 372.6us; measured HW via
burst-slope tracks sim at a consistent ~1.65x in this environment).
"""

import numpy as np
import ml_dtypes

import concourse.bacc as bacc
import concourse.mybir as mybir
import concourse.tile as tile
from concourse import bass_utils

F32 = mybir.dt.float32
F32R = mybir.dt.float32r
BF16 = mybir.dt.bfloat16
FP8 = mybir.dt.float8e4
AF = mybir.ActivationFunctionType
MUL = mybir.AluOpType.mult
DR = mybir.MatmulPerfMode.DoubleRow
NP_BF16 = ml_dtypes.bfloat16
NP_FP8 = ml_dtypes.float8_e4m3fn

# fp8 q/k for the logits matmul (DoubleRow, 2x PE rate). The host permutes
# Wq/Wk output channels so the projection writes the [32-part x 2-ktile]
# layout DoubleRow wants; logits are invariant to a shared q/k channel
# permutation. Measured end-to-end error ~1.2e-2 vs the 2e-2 gate.
USE_FP8_QK = True

P = 128
C = 1024
D = 64
LQ = 1024
LK = 4096
CS = 256          # channels per core (4 heads)
NH = 4            # heads per core
SCALE = 1.0 / 8.0
BLK = 512         # Lk rows per processing block
NBLK = LK // BLK  # 8
NCH = LK // P     # 32 Lk chunks of 128
QB = 512          # Lq rows per Q-projection block


def build_nc():
    nc = bacc.Bacc("TRN2", target_bir_lowering=False, debug=False, num_devices=8)
    qt_d = nc.dram_tensor("QT", [C, LQ], BF16, kind="ExternalInput").ap()
    kt_d = nc.dram_tensor("KT", [C, LK], BF16, kind="ExternalInput").ap()
    vt_d = nc.dram_tensor("VT", [C, LK], BF16, kind="ExternalInput").ap()
    vbias_d = nc.dram_tensor("vbias", [P, NCH], F32, kind="ExternalInput").ap()
    wq_d = nc.dram_tensor("wq", [C, CS], BF16, kind="ExternalInput").ap()
    wk_d = nc.dram_tensor("wk", [C, CS], BF16, kind="ExternalInput").ap()
    wv_d = nc.dram_tensor("wv", [C, CS], BF16, kind="ExternalInput").ap()
    wo_d = nc.dram_tensor("wo", [CS, C], BF16, kind="ExternalInput").ap()
    bq_d = nc.dram_tensor("bq", [P, 2], F32, kind="ExternalInput").ap()
    bk_d = nc.dram_tensor("bk", [P, 2], F32, kind="ExternalInput").ap()
    out_d = nc.dram_tensor("out", [2, LQ, C], BF16, kind="ExternalOutput").ap()

    with tile.TileContext(nc) as tc:
        with (
            tc.tile_pool(name="pers", bufs=1) as pers,
            tc.tile_pool(name="sb", bufs=1) as sb,
            tc.tile_pool(name="ps", bufs=2, space="PSUM") as ps,
        ):
            # ---- constants ----
            ones_f32 = pers.tile([P, 1], F32)
            nc.gpsimd.memset(ones_f32, 1.0)
            ones_r = pers.tile([1, D], F32R)
            nc.vector.tensor_copy(ones_r, ones_f32[0:1, :].to_broadcast([1, D]))
            vbias_sb = pers.tile([P, NCH], F32)
            nc.sync.dma_start(vbias_sb, vbias_d)
            bq_sb = pers.tile([P, 2], F32)
            nc.sync.dma_start(bq_sb, bq_d)
            bk_sb = pers.tile([P, 2], F32)
            nc.sync.dma_start(bk_sb, bk_d)

            # ---- persistent activations ----
            QKDT = FP8 if USE_FP8_QK else BF16
            qT = pers.tile([P, 2, LQ], QKDT)       # [ch%128, ch//128, Lq]
            kT = pers.tile([P, 2, LK], QKDT)
            if USE_FP8_QK:
                # DoubleRow layout: [32*hl+pp, hh, t, L]; head h=(hh=h//2,
                # hl=h%2) strip sits at base partition 32*hl (0/32), with
                # channel d = 32*t + pp. Filled by SBUF->SBUF DMA remap.
                qT8 = pers.tile([D, 2, 2, LQ], FP8)
                kT8 = pers.tile([D, 2, 2, LK], FP8)

                def remap(dst8, stage, lo, hi):
                    # 4 simple DMAs (3-dim APs): strip (hl,t) of the stage
                    # moves to partitions 32*hl..+32, ktile column t
                    for hl in range(2):
                        for t in range(2):
                            p0 = hl * 64 + t * 32
                            nc.gpsimd.dma_start(
                                dst8[hl * 32 : (hl + 1) * 32, :, t, lo:hi],
                                stage[p0 : p0 + 32, :, lo:hi],
                            )
            v65 = pers.tile([P, NCH, NH, D + 1], BF16)  # [k%128, chunk, h, v|1]
            attnT = pers.tile([P, 2, LQ], BF16)
            wqT = pers.tile([P, 8, CS], BF16)      # [cin%128, cin//128, cout]
            wkT = pers.tile([P, 8, CS], BF16)
            wvT = pers.tile([P, 8, CS], BF16)
            woT = pers.tile([P, 2, C], BF16)       # [ch%128, ch//128, cout]
            RING = 24
            e2ring = pers.tile([P, RING, LQ], BF16)  # head-2 attn weights

            # ones column of v65 (softmax denominator trick)
            with nc.allow_low_precision(reason="ones column exact in bf16"):
                nc.vector.tensor_copy(
                    v65[:, :, :, D].rearrange("p a b -> p (a b)"),
                    ones_f32.to_broadcast([P, NCH * NH]),
                )

            def issue_kv(blk):
                kx = sb.tile([P, 8, BLK], BF16, tag="kx", bufs=2)
                nc.sync.dma_start(
                    kx,
                    kt_d[:, blk * BLK : (blk + 1) * BLK].rearrange(
                        "(i p) r -> p i r", p=P
                    ),
                )
                vx = sb.tile([P, 8, BLK], BF16, tag="vx", bufs=2)
                nc.sync.dma_start(
                    vx,
                    vt_d[:, blk * BLK : (blk + 1) * BLK].rearrange(
                        "(i p) r -> p i r", p=P
                    ),
                )
                return kx, vx

            # ---- DMA order: wk, K+V block 0, wq, wv, Q halves ----
            nc.sync.dma_start(wkT, wk_d.rearrange("(i p) o -> p i o", p=P))
            pend = {0: issue_kv(0)}
            nc.sync.dma_start(wqT, wq_d.rearrange("(i p) o -> p i o", p=P))
            nc.sync.dma_start(wvT, wv_d.rearrange("(i p) o -> p i o", p=P))

            def q_proj():
                # emitted AFTER block-0 projections so PE's in-order queue
                # starts on K/V work (whose DMAs land first)
                for h in range(2):
                    qx = sb.tile([P, 8, QB], BF16, tag="qx", bufs=2)
                    nc.sync.dma_start(
                        qx,
                        qt_d[:, h * QB : (h + 1) * QB].rearrange(
                            "(i p) r -> p i r", p=P
                        ),
                    )
                    for mt in range(2):
                        pq = ps.tile([P, LQ], F32, tag="pl", bufs=2, name="pq")
                        for i in range(8):
                            nc.tensor.matmul(
                                pq[:, 0:QB], wqT[:, i, mt * P : (mt + 1) * P],
                                qx[:, i, :], start=(i == 0), stop=(i == 7),
                            )
                        with nc.allow_low_precision(reason="bf16 activations"):
                            nc.vector.tensor_scalar_add(
                                qT[:, mt, h * QB : (h + 1) * QB], pq[:, 0:QB],
                                bq_sb[:, mt : mt + 1],
                            )
                    if USE_FP8_QK:
                        remap(qT8, qT, h * QB, (h + 1) * QB)

            # ---- attention helpers ----
            def attn_logits(h, c, dst=None):
                ht, hp = h // 2, (h % 2) * D
                pl = ps.tile([P, LQ], F32, tag="pl", bufs=2, name="pl")
                hh, hl = h // 2, (h % 2) * 32
                for n in range(2):
                    if USE_FP8_QK:
                        nc.tensor.matmul(
                            pl[:, n * 512 : (n + 1) * 512],
                            kT8[hl : hl + 32, hh, :, c * P : (c + 1) * P],
                            qT8[hl : hl + 32, hh, :, n * 512 : (n + 1) * 512],
                            start=True, stop=True, perf_mode=DR,
                        )
                    else:
                        nc.tensor.matmul(
                            pl[:, n * 512 : (n + 1) * 512],
                            kT[hp : hp + D, ht, c * P : (c + 1) * P],
                            qT[hp : hp + D, ht, n * 512 : (n + 1) * 512],
                            start=True, stop=True, tile_position=(hp, 0),
                        )
                eT = dst if dst is not None else sb.tile(
                    [P, LQ], BF16, tag="exp", bufs=4, name="eT"
                )
                with nc.allow_low_precision(reason="attn weights bf16"):
                    nc.scalar.activation(
                        eT, pl, AF.Exp, bias=vbias_sb[:, c : c + 1], scale=SCALE
                    )
                return eT

            def attn_av(h, c, po, eT):
                for n in range(2):
                    nc.tensor.matmul(
                        po[0 : D + 1, n * 512 : (n + 1) * 512],
                        v65[:, c, h, :],
                        eT[:, n * 512 : (n + 1) * 512],
                        start=(c == 0), stop=(c == NCH - 1),
                    )

            def attn_chunk(h, c, po):
                attn_av(h, c, po, attn_logits(h, c))

            def attn_finish(h, po):
                ht, hp = h // 2, (h % 2) * D
                rec = sb.tile([1, LQ], F32R, tag="rec", bufs=2, name="rec")
                with nc.allow_low_precision(reason="softmax denom reciprocal"):
                    nc.vector.reciprocal(rec, po[D : D + 1, :])
                pb = ps.tile([P, LQ], F32, tag="pl", bufs=2, name="pb")
                for n in range(2):
                    nc.tensor.matmul(
                        pb[0:D, n * 512 : (n + 1) * 512],
                        ones_r,
                        rec[:, n * 512 : (n + 1) * 512],
                        start=True, stop=True,
                    )
                bc = sb.tile([D, LQ], F32, tag="bc", bufs=2, name="bc")
                nc.scalar.copy(bc, pb[0:D, :])
                with nc.allow_low_precision(reason="bf16 attn output"):
                    nc.vector.tensor_tensor(
                        attnT[hp : hp + D, ht, :], po[0:D, :], bc, MUL
                    )

            def out_proj_m(kc, m, eng=0, split=False):
                pw = ps.tile([P, C], F32, tag="pl", bufs=2, name="pw")
                for n in range(2):
                    if split:  # per-head halves (even head's half ready first)
                        for hp in range(2):
                            nc.tensor.matmul(
                                pw[:, n * 512 : (n + 1) * 512],
                                attnT[hp * D : (hp + 1) * D, kc,
                                      m * P : (m + 1) * P],
                                woT[hp * D : (hp + 1) * D, kc,
                                    n * 512 : (n + 1) * 512],
                                start=(hp == 0), stop=(hp == 1),
                                tile_position=(hp * D, 0),
                            )
                    else:
                        nc.tensor.matmul(
                            pw[:, n * 512 : (n + 1) * 512],
                            attnT[:, kc, m * P : (m + 1) * P],
                            woT[:, kc, n * 512 : (n + 1) * 512],
                            start=True, stop=True,
                        )
                ob = sb.tile([P, C], BF16, tag="ob", bufs=4)
                with nc.allow_low_precision(reason="bf16 partial out"):
                    if eng:
                        nc.scalar.copy(ob, pw)
                    else:
                        nc.vector.tensor_copy(ob, pw)
                nc.sync.dma_start(out_d[kc, m * P : (m + 1) * P, :], ob)

            # ---- K/V projection blocks; heads 0-1 stream behind them ----
            po0 = ps.tile([P, LQ], F32, tag="acc", bufs=2, name="po0")
            po1 = ps.tile([P, LQ], F32, tag="acc", bufs=2, name="po1")
            sprev = None

            def stream_attn(blk):
                nonlocal sprev
                for cc in range(4):
                    c = blk * 4 + cc
                    e0 = attn_logits(0, c)
                    e1 = attn_logits(1, c)
                    if sprev is not None:
                        attn_av(0, sprev[0], po0, sprev[1])
                        attn_av(1, sprev[0], po1, sprev[2])
                    sprev = (c, e0, e1)
                    if cc < 3:  # precompute head-2 exps (3 per block)
                        c2 = blk * 3 + cc
                        attn_logits(2, c2, dst=e2ring[:, c2, :])

            for blk in range(NBLK):
                kx, vx = pend.pop(blk)
                if blk == NBLK - 1:
                    nc.sync.dma_start(
                        woT, wo_d.rearrange("(t p) c -> p t c", p=P)
                    )
                for mt in range(2):
                    pk = ps.tile([P, LQ], F32, tag="pl", bufs=2, name="pk")
                    for i in range(8):
                        nc.tensor.matmul(
                            pk[:, 0:BLK], wkT[:, i, mt * P : (mt + 1) * P],
                            kx[:, i, :], start=(i == 0), stop=(i == 7),
                        )
                    with nc.allow_low_precision(reason="bf16 activations"):
                        nc.vector.tensor_scalar_add(
                            kT[:, mt, blk * BLK : (blk + 1) * BLK], pk[:, 0:BLK],
                            bk_sb[:, mt : mt + 1],
                        )
                if USE_FP8_QK:
                    remap(kT8, kT, blk * BLK, (blk + 1) * BLK)
                for pr in range(2):  # pairs of 128-row chunks
                    pv = ps.tile([P, LQ], F32, tag="pl", bufs=2, name="pv")
                    for half in range(2):
                        lo = pr * 256 + half * P
                        for i in range(8):
                            nc.tensor.matmul(
                                pv[:, half * 256 : (half + 1) * 256],
                                vx[:, i, lo : lo + P], wvT[:, i, :],
                                start=(i == 0), stop=(i == 7),
                            )
                    c0 = blk * 4 + pr * 2
                    with nc.allow_low_precision(reason="v bf16"):
                        nc.vector.tensor_copy(
                            v65[:, c0 : c0 + 2, :, 0:D],
                            pv[:, 0:BLK].rearrange("p (r h d) -> p r h d", r=2, d=D),
                        )
                if blk == 0:
                    q_proj()
                if blk + 1 < NBLK:
                    pend[blk + 1] = issue_kv(blk + 1)
                # attention one block behind: hides proj->remap latency
                if blk > 0:
                    stream_attn(blk - 1)
            stream_attn(NBLK - 1)
            attn_av(0, sprev[0], po0, sprev[1])
            attn_av(1, sprev[0], po1, sprev[2])
            attn_finish(0, po0)
            attn_finish(1, po1)

            # ---- heads 2-3 (kT/v65 complete); head-pair-0 out streams ----
            po2 = ps.tile([P, LQ], F32, tag="acc", bufs=2, name="po2")
            po3 = ps.tile([P, LQ], F32, tag="acc", bufs=2, name="po3")
            # one-chunk software pipeline: emit logits(c) before attn@v(c-1)
            # so the exp stream never waits on PE
            prev = None
            for c in range(NCH):
                e3 = attn_logits(3, c)
                e2 = e2ring[:, c, :] if c < RING else attn_logits(2, c)
                if prev is not None:
                    attn_av(2, prev[0], po2, prev[1])
                    attn_av(3, prev[0], po3, prev[2])
                prev = (c, e2, e3)
                if c % 4 == 2:
                    out_proj_m(0, c // 4)
            attn_av(2, prev[0], po2, prev[1])
            attn_av(3, prev[0], po3, prev[2])
            attn_finish(2, po2)
            attn_finish(3, po3)
            for m in range(8):
                out_proj_m(1, m, eng=(m % 2))

    nc.compile()
    return nc


_NC = None


def _get_nc():
    global _NC
    if _NC is None:
        _NC = build_nc()
    return _NC


def _bf(x):
    return np.ascontiguousarray(x.astype(NP_BF16))




def shard_inputs(Q, K_in, V_in, V_bias, Wq_w, Wq_b, Wk_w, Wk_b, Wv_w, Wv_b, Wo_w, Wo_b):
    """Build the 8 per-core input dicts (host pre-transposes + casts bf16)."""
    per_batch = []
    for b in range(2):
        per_batch.append({
            "QT": _bf(np.asarray(Q[b]).T),
            "KT": _bf(np.asarray(K_in[b]).T),
            "VT": _bf(np.asarray(V_in[b]).T),
            "vbias": np.ascontiguousarray(np.asarray(V_bias[b]).reshape(NCH, P).T),
        })
    in_maps = []
    for core in range(8):
        b, g = core // 4, core % 4
        gs, ge = g * CS, (g + 1) * CS
        in_maps.append({
            **per_batch[b],
            "wq": _bf(np.asarray(Wq_w)[gs:ge].T),
            "wk": _bf(np.asarray(Wk_w)[gs:ge].T),
            "wv": _bf(np.asarray(Wv_w)[gs:ge].T),
            "wo": _bf(np.asarray(Wo_w)[:, gs:ge].T),
            "bq": np.ascontiguousarray(np.asarray(Wq_b)[gs:ge].reshape(2, P).T),
            "bk": np.ascontiguousarray(np.asarray(Wk_b)[gs:ge].reshape(2, P).T),
        })
    return in_maps


def combine_outputs(results, Wv_b, Wo_w, Wo_b):
    """Sum the 2x4 head-group partials per batch; add output bias and the
    folded V-projection bias (attention weights sum to 1)."""
    bias = Wo_b + Wv_b @ Wo_w.T
    outs = np.stack(
        [np.asarray(r["out"]).astype(np.float32) for r in results]
    ).reshape(2, 4 * 2, LQ, C)
    return (outs.sum(axis=1) + bias[None, None, :]).astype(np.float32)


def kernel(**inputs):
    nc = _get_nc()
    in_maps = shard_inputs(**inputs)
    res = bass_utils.run_bass_kernel_spmd(nc, in_maps, core_ids=list(range(8)))
    return combine_outputs(
        res.results,
        np.asarray(inputs["Wv_b"]),
        np.asarray(inputs["Wo_w"]),
        np.asarray(inputs["Wo_b"]),
    )


if __name__ == "__main__":
    rng = np.random.default_rng(0)
    ins = {
        "Q": rng.standard_normal((2, LQ, C), dtype=np.float32),
        "K_in": rng.standard_normal((2, LK, C), dtype=np.float32),
        "V_in": rng.standard_normal((2, LK, C), dtype=np.float32),
        "V_bias": rng.standard_normal((2, LK)).astype(np.float32),
        **{
            f"W{x}_w": (rng.standard_normal((C, C)) * 0.03).astype(np.float32)
            for x in "qkvo"
        },
        **{
            f"W{x}_b": (rng.standard_normal(C) * 0.03).astype(np.float32)
            for x in "qkvo"
        },
    }
    out = kernel(**ins)
    print("ok", out.shape, out.dtype)
